# revision 12
# baseline (speedup 1.0000x reference)
"""Trainium2 Bass kernel for a GAT-style attention head.

Reference computation (B=1, C=512, N=8192, F=256):
    seq_fts = einsum('bcn,fc->bfn', x, W1)                  # [1,F,N]
    f1 = seq_fts . w21 + b21 ;  f2 = seq_fts . w22 + b22    # [1,N]
    logits[i,j] = f1[j] + f2[i]  masked by adj>0 (else -1e9)
    logits = leaky_relu(logits, 0.01)
    coefs = softmax(logits, axis=1)        # normalises over i for each j
    ret[i,f] = sum_j coefs[i,j]*seq_fts[f,j] + bias[f]
    out = elu(ret).transpose -> [1,F,N]

Distribution: shard rows i across 8 NeuronCores (1024 rows each).  The
softmax denominator D[j] = sum_i E[i,j] (E = exp of masked leaky-relu
logits) is indexed by the *contracted* axis j, so each core computes a
partial D over its rows, a 32KB AllReduce produces the full D, and 1/D
is folded into the seq_fts columns before the local matmul
    out[f, i_blk] = sum_j (seqT[j,f]/D[j]) * E[j, i_blk].

Per-core pipeline (E kept transposed, [j on partitions, i free], fp16):
  - seqT (=seq_fts^T) and f1 computed on the core's own n-block in fp32,
    then AllGathered (seqT as fp16).
  - logits tile [128j, 1024i]: ACT Lrelu(f2_bcast + f1[j] per-partition
    bias) -> fp16; DVE adds the {0,-1e4} additive mask tile (equivalent
    to the reference's -1e9 pre-relu mask: exp underflows to 0 either
    way); ACT Exp with accum_out giving the partial D for those 128 j.
  - 4-way chunked AllReduce of D overlaps the elementwise phase; fp16
    matmuls accumulate out[f,i] over the 64 j-tiles in PSUM.
  - epilogue: bias add + ELU via relu(x)+exp(min(x,0))-1.
"""

import os
import sys

if "/opt/trn_rl_repo" not in sys.path:
    sys.path.insert(0, "/opt/trn_rl_repo")

import numpy as np

import concourse.bass as bass
import concourse.tile as tile
from concourse import bacc, mybir

F32 = mybir.dt.float32
F16 = mybir.dt.float16

B, C, N, F = 1, 512, 8192, 256
NCORES = 8
NB = N // NCORES          # rows per core (i block)
P = 128
NJT = N // P              # 64 j tiles
NS = NB // P              # 8 n sub-tiles per core
CO = C // P               # 4 contraction tiles for seq_fts
NCHUNK = 4                # allreduce chunks
CH = [20, 20, 20, 4]      # j-tiles per chunk (small tail chunk)
CH0 = [0, 20, 40, 60]     # chunk start offsets
CJT = 20                  # max chunk size (tile sizing)
MM_N = 512                # max moving free dim for 16-bit matmul

AF = mybir.ActivationFunctionType
OP = mybir.AluOpType

_PROGRAM_CACHE = {}
LAST_RESULTS = None       # BassKernelResults of the most recent run (for test.py)


def _build_program(b21f: float, b22f: float):
    nc = bacc.Bacc("TRN2", target_bir_lowering=False, debug=False,
                   num_devices=NCORES)

    # ---- per-core external inputs -------------------------------------
    xb_t = nc.dram_tensor("xb", [C, NB], F32, kind="ExternalInput")
    xh_t = nc.dram_tensor("xh", [C, N], F16, kind="ExternalInput")
    xl_t = nc.dram_tensor("xl", [C, N], F16, kind="ExternalInput")
    w1t_t = nc.dram_tensor("w1t", [C, F], F32, kind="ExternalInput")
    w21_t = nc.dram_tensor("w21", [1, F], F32, kind="ExternalInput")
    w22_t = nc.dram_tensor("w22", [1, F], F32, kind="ExternalInput")
    bias_t = nc.dram_tensor("bias", [F], F32, kind="ExternalInput")
    mk_t = nc.dram_tensor("mk", [N, NB], F16, kind="ExternalInput")
    out_t = nc.dram_tensor("outb", [F, NB], F32, kind="ExternalOutput")

    groups = [list(range(NCORES))]

    with tile.TileContext(nc) as tc:
        with tc.tile_pool(name="dram", bufs=1, space="DRAM") as dram:
            f1_dram = dram.tile([N], F32, name="f1_dram")
            ag2_in = dram.tile([NB * F], F16, name="ag2_in")
            ag2_out = dram.tile([N * F], F16, name="ag2_out",
                                addr_space="Shared")
            f2tmp = dram.tile([NB], F32, name="f2tmp")
            ar_in = [dram.tile([P * CH[k]], F32, name=f"ar_in{k}")
                     for k in range(NCHUNK)]
            ar_out = [dram.tile([P * CH[k]], F32, name=f"ar_out{k}",
                                addr_space="Shared") for k in range(NCHUNK)]

            # ---------- persistent SBUF ----------
            with tc.tile_pool(name="persist", bufs=1) as persist:
                seqt = persist.tile([P, NJT, F], F16, name="seqt")
                f2b = persist.tile([P, NB], F32, name="f2b")
                f1col = persist.tile([P, NJT], F32, name="f1col")
                bias_sb = persist.tile([P, F // P], F32, name="bias_sb")
                ident = persist.tile([P, P], F32, name="ident")

                # ---------- phase 0 ----------
                with tc.tile_pool(name="p0", bufs=1) as p0, \
                     tc.tile_pool(name="p0s", bufs=3) as p0s, \
                     tc.tile_pool(name="p0ps", bufs=2, space="PSUM") as p0ps:
                    x_sb = p0.tile([P, CO, NB], F32, name="x_sb")
                    nc.sync.dma_start(
                        x_sb[:],
                        xb_t.ap().rearrange("(co ci) n -> ci co n", ci=P))
                    w1t_sb = p0.tile([P, CO, F], F32, name="w1t_sb")
                    nc.sync.dma_start(
                        w1t_sb[:],
                        w1t_t.ap().rearrange("(co ci) f -> ci co f", ci=P))
                    w21b = p0.tile([P, F], F32, name="w21b")
                    nc.sync.dma_start(w21b[:],
                                      w21_t.ap()[0:1, :].to_broadcast((P, F)))
                    w22b = p0.tile([P, F], F32, name="w22b")
                    nc.sync.dma_start(w22b[:],
                                      w22_t.ap()[0:1, :].to_broadcast((P, F)))
                    nc.sync.dma_start(
                        bias_sb[:],
                        bias_t.ap().rearrange("(ft fi) -> fi ft", fi=P))

                    # u1/u2 = W1^T w21 / w22  (fp32, per c partition)
                    u_sb = p0.tile([P, CO, 2], F32, name="u_sb")
                    for co in range(CO):
                        tu = p0.tile([P, F], F32, name="tu", tag="tu")
                        nc.vector.tensor_tensor(tu[:], w1t_sb[:, co, :],
                                                w21b[:], OP.mult)
                        nc.vector.tensor_reduce(u_sb[:, co, 0:1], tu[:],
                                                mybir.AxisListType.X, OP.add)
                        tv = p0.tile([P, F], F32, name="tv", tag="tv")
                        nc.vector.tensor_tensor(tv[:], w1t_sb[:, co, :],
                                                w22b[:], OP.mult)
                        nc.vector.tensor_reduce(u_sb[:, co, 1:2], tv[:],
                                                mybir.AxisListType.X, OP.add)
                    uh_sb = p0.tile([P, CO, 2], F16, name="uh_sb")
                    nc.vector.tensor_copy(uh_sb[:], u_sb[:])

                    # f2 (own block) exact: u2^T x_own, fp32 matmul
                    f2ps = p0ps.tile([1, NB], F32, name="f2ps", bufs=1)
                    for ih in range(2):
                        for co in range(CO):
                            nc.tensor.matmul(
                                f2ps[:, ih * MM_N:(ih + 1) * MM_N],
                                lhsT=u_sb[:, co, 1:2],
                                rhs=x_sb[:, co, ih * MM_N:(ih + 1) * MM_N],
                                start=(co == 0), stop=(co == CO - 1))
                    f2row = p0.tile([1, NB], F32, name="f2row")
                    nc.vector.tensor_scalar_add(f2row[:], f2ps[:], b22f)
                    nc.sync.dma_start(f2tmp[:].rearrange("n -> () n"),
                                      f2row[:])
                    # f2 broadcast for the logits activation
                    nc.sync.dma_start(
                        f2b[:],
                        f2tmp[None, :].to_broadcast((P, NB)))

                    # f1 (all n) ~fp16: uh1^T (xh + xl), streamed over n
                    NF1 = 16
                    F1C = N // NF1   # 512 wide chunks
                    for q in range(NF1):
                        xht = p0s.tile([P, CO, F1C], F16, name="xht",
                                       tag="xht", bufs=2)
                        nc.sync.dma_start(
                            xht[:],
                            xh_t.ap().rearrange("(co ci) n -> ci co n",
                                                ci=P)[:, :,
                                                      q * F1C:(q + 1) * F1C])
                        xlt = p0s.tile([P, CO, F1C], F16, name="xlt",
                                       tag="xlt", bufs=2)
                        nc.sync.dma_start(
                            xlt[:],
                            xl_t.ap().rearrange("(co ci) n -> ci co n",
                                                ci=P)[:, :,
                                                      q * F1C:(q + 1) * F1C])
                        f1ps = p0ps.tile([1, F1C], F32, name="f1ps",
                                         tag="f1ps")
                        for co in range(CO):
                            nc.tensor.matmul(
                                f1ps[:], lhsT=uh_sb[:, co, 0:1],
                                rhs=xht[:, co, :],
                                start=(co == 0), stop=False)
                        for co in range(CO):
                            nc.tensor.matmul(
                                f1ps[:], lhsT=uh_sb[:, co, 0:1],
                                rhs=xlt[:, co, :],
                                start=False, stop=(co == CO - 1))
                        f1c_sb = p0s.tile([1, F1C], F32, name="f1c_sb",
                                          tag="f1c_sb")
                        nc.vector.tensor_scalar_add(f1c_sb[:], f1ps[:], b21f)
                        nc.sync.dma_start(
                            f1_dram[q * F1C:(q + 1) * F1C].rearrange(
                                "n -> () n"),
                            f1c_sb[:])

                    # f1col[jp, jt] = f1[jt*128+jp] via PE transpose
                    make_ident_f32(nc, ident)
                    t64 = p0.tile([NJT, P], F32, name="t64")
                    nc.sync.dma_start(
                        t64[:], f1_dram.rearrange("(jt jp) -> jt jp", jp=P))
                    tps = p0ps.tile([P, NJT], F32, name="tps", bufs=1)
                    nc.tensor.matmul(tps[:], lhsT=t64[:],
                                     rhs=ident[:NJT, :NJT],
                                     is_transpose=True, start=True, stop=True)
                    nc.scalar.copy(f1col[:], tps[:])

                    # seqT (own block, fp16): W1h^T (xh_own + xl_own)
                    xho = p0.tile([P, CO, NB], F16, name="xho")
                    nc.vector.tensor_copy(xho[:], x_sb[:])
                    w1h = p0.tile([P, CO, F], F16, name="w1h")
                    nc.vector.tensor_copy(w1h[:], w1t_sb[:])
                    seqtown = p0.tile([P, NS, F], F16, name="seqtown")
                    for ns in range(NS):
                        sps = p0ps.tile([P, F], F32, name="sps", tag="sps")
                        for co in range(CO):
                            nc.tensor.matmul(
                                sps[:],
                                lhsT=xho[:, co, ns * P:(ns + 1) * P],
                                rhs=w1h[:, co, :],
                                start=(co == 0), stop=(co == CO - 1))
                        nc.scalar.copy(seqtown[:, ns, :], sps[:])
                    # seqT-own dump (ci-major, contiguous) + AllGather
                    nc.sync.dma_start(
                        ag2_in.rearrange("(ci ns f) -> ci ns f",
                                         ci=P, ns=NS),
                        seqtown[:])
                    nc.gpsimd.collective_compute(
                        "AllGather", OP.bypass, replica_groups=groups,
                        ins=[ag2_in.opt()], outs=[ag2_out.opt()])
                    # seqT gather: per source core block
                    for b in range(NCORES):
                        src = ag2_out.rearrange(
                            "(b ci ns f) -> b ci ns f", b=NCORES, ci=P, ns=NS)
                        nc.sync.dma_start(seqt[:, b * NS:(b + 1) * NS, :],
                                          src[b])

                # ---------- main loop ----------
                with tc.tile_pool(name="etpool", bufs=1) as etp, \
                     tc.tile_pool(name="stream", bufs=3) as stream, \
                     tc.tile_pool(name="dtiles", bufs=1) as dtiles, \
                     tc.tile_pool(name="outps", bufs=1, space="PSUM") as outps, \
                     tc.tile_pool(name="epil", bufs=1) as epil:

                    et_c = [etp.tile([P, CH[k], NB], F16, name=f"et{k}",
                                     tag=f"et{k}") for k in range(NCHUNK)]
                    out_ps = [outps.tile([P, MM_N], F32, name=f"out_ps{q}",
                                         tag=f"out_ps{q}")
                              for q in range(4)]
                    dp_c = [dtiles.tile([P, CH[k]], F32, name=f"dp{k}",
                                        tag=f"dp{k}") for k in range(NCHUNK)]
                    inv_c = [dtiles.tile([P, CH[k]], F32, name=f"inv{k}",
                                         tag=f"inv{k}") for k in range(NCHUNK)]

                    for k in range(NCHUNK):
                        for jl in range(CH[k]):
                            jt = CH0[k] + jl
                            mk_tile = stream.tile([P, NB], F16, name="mk_tile",
                                                  tag="mk")
                            nc.sync.dma_start(
                                mk_tile[:],
                                mk_t.ap()[jt * P:(jt + 1) * P, :])
                            if jt % 2 == 0:
                                # ACT path: fused add+leaky-relu on ScalarE
                                ltile = stream.tile([P, NB], F16,
                                                    name="ltile", tag="lt", bufs=2)
                                nc.scalar.activation(
                                    ltile[:], f2b[:], AF.Prelu,
                                    bias=f1col[:, jt:jt + 1], scale=1.0,
                                    alpha=0.01)
                            else:
                                # DVE path: v fp32 exact, then STT leaky-relu
                                vt = stream.tile([P, NB], F32, name="vt",
                                                 tag="vt", bufs=2)
                                nc.vector.tensor_scalar_add(
                                    vt[:], f2b[:], f1col[:, jt:jt + 1])
                                ltile = stream.tile([P, NB], F16,
                                                    name="ltile2", tag="lt2", bufs=2)
                                nc.vector.scalar_tensor_tensor(
                                    ltile[:], vt[:], 0.01, vt[:],
                                    OP.mult, OP.max)
                            l2 = stream.tile([P, NB], F16, name="l2", tag="l2")
                            nc.vector.tensor_tensor(l2[:], ltile[:],
                                                    mk_tile[:], OP.add)
                            nc.scalar.activation(
                                et_c[k][:, jl, :], l2[:], AF.Exp,
                                accum_out=dp_c[k][:, jl:jl + 1])

                        # chunk-k AllReduce of partial D
                        nc.sync.dma_start(
                            ar_in[k].rearrange("(jp jl) -> jp jl", jp=P),
                            dp_c[k][:])
                        nc.gpsimd.collective_compute(
                            "AllReduce", OP.add, replica_groups=groups,
                            ins=[ar_in[k].opt()], outs=[ar_out[k].opt()])
                        dsum = dtiles.tile([P, CH[k]], F32, name=f"dsum{k}",
                                           tag=f"dsum{k}")
                        nc.sync.dma_start(
                            dsum[:],
                            ar_out[k].rearrange("(jp jl) -> jp jl", jp=P))
                        nc.vector.reciprocal(inv_c[k][:], dsum[:])

                        # matmuls for this chunk
                        for jl in range(CH[k]):
                            jt = CH0[k] + jl
                            seqd = stream.tile([P, F], F16, name="seqd",
                                               tag="seqd")
                            nc.vector.tensor_scalar_mul(
                                seqd[:], seqt[:, jt, :],
                                inv_c[k][:, jl:jl + 1])
                            for fi in range(2):
                                for ih in range(2):
                                    nc.tensor.matmul(
                                        out_ps[fi * 2 + ih][:],
                                        lhsT=seqd[:, fi * P:(fi + 1) * P],
                                        rhs=et_c[k][:, jl,
                                               ih * MM_N:(ih + 1) * MM_N],
                                        start=(jt == 0), stop=(jt == NJT - 1))

                    # ---------- epilogue: bias + ELU ----------
                    for fi in range(2):
                        for ih in range(2):
                            ps = out_ps[fi * 2 + ih]
                            t = epil.tile([P, MM_N], F32, name="t", tag="ep_t")
                            nc.scalar.activation(t[:], ps[:], AF.Identity,
                                                 bias=bias_sb[:, fi:fi + 1],
                                                 scale=1.0)
                            r = epil.tile([P, MM_N], F32, name="r", tag="ep_r")
                            nc.vector.tensor_scalar_max(r[:], t[:], 0.0)
                            m = epil.tile([P, MM_N], F32, name="m", tag="ep_m")
                            nc.vector.tensor_scalar_min(m[:], t[:], 0.0)
                            e = epil.tile([P, MM_N], F32, name="e", tag="ep_e")
                            nc.scalar.activation(e[:], m[:], AF.Exp)
                            o = epil.tile([P, MM_N], F32, name="o", tag="ep_o")
                            nc.vector.scalar_tensor_tensor(
                                o[:], e[:], -1.0, r[:], OP.add, OP.add)
                            nc.sync.dma_start(
                                out_t.ap()[fi * P:(fi + 1) * P,
                                           ih * MM_N:(ih + 1) * MM_N], o[:])

    nc.compile()
    return nc


def make_ident_f32(nc, ident):
    nc.gpsimd.memset(ident[:], 0.0)
    nc.gpsimd.affine_select(
        out=ident[:], in_=ident[:], compare_op=OP.not_equal, fill=1.0,
        base=0, pattern=[[-1, P]], channel_multiplier=1)


def kernel(x, adj, W1, w21, b21, w22, b22, bias):
    global LAST_RESULTS
    from concourse.bass_utils import run_bass_kernel_spmd

    x = np.asarray(x)
    adj = np.asarray(adj)
    W1 = np.asarray(W1, dtype=np.float32)
    w21 = np.asarray(w21, dtype=np.float32)
    w22 = np.asarray(w22, dtype=np.float32)
    bias = np.asarray(bias, dtype=np.float32)
    b21f = float(np.asarray(b21))
    b22f = float(np.asarray(b22))

    key = (b21f, b22f)
    if key not in _PROGRAM_CACHE:
        _PROGRAM_CACHE[key] = _build_program(b21f, b22f)
    nc = _PROGRAM_CACHE[key]

    w1t = np.ascontiguousarray(W1.T)                      # [C, F]
    x2d = np.ascontiguousarray(x[0], dtype=np.float32)    # [C, N]
    xh = x2d.astype(np.float16)
    xl = (x2d - xh.astype(np.float32)).astype(np.float16)
    in_maps = []
    for c in range(NCORES):
        blk = slice(c * NB, (c + 1) * NB)
        xb = np.ascontiguousarray(x[0, :, blk], dtype=np.float32)
        # additive mask, transposed: [j, i_local] fp16 {0, -1e4}
        mk = ((adj[0, blk, :].T.astype(np.float32) - 1.0)
              * 1.0e4).astype(np.float16)
        in_maps.append({
            "xb": xb,
            "xh": xh,
            "xl": xl,
            "w1t": w1t,
            "w21": w21.reshape(1, F),
            "w22": w22.reshape(1, F),
            "bias": bias,
            "mk": mk,
        })

    trace = os.environ.get("BASS_KERNEL_TRACE") == "1"
    kwargs = {}
    if trace:
        _install_ntff_hook()
        import concourse.bass_utils as bu
        bu.upload_artifacts = lambda d: d          # no S3 in this sandbox
        kwargs = dict(trace=True, trace_cores=list(range(NCORES)),
                      tmpdir=os.environ.get("BASS_KERNEL_TRACE_DIR"))

    res = run_bass_kernel_spmd(nc, in_maps, core_ids=list(range(NCORES)),
                               **kwargs)
    LAST_RESULTS = res

    out = np.empty((B, F, N), dtype=np.float32)
    for c in range(NCORES):
        out[0, :, c * NB:(c + 1) * NB] = res.results[c]["outb"]
    return out


def _install_ntff_hook():
    """Register the axon NTFF profiling hook (missing antenv.axon_hooks)."""
    import types
    import contextlib
    import ctypes

    if "antenv.axon_hooks" in sys.modules:
        return
    so_path = "/opt/axon/libaxon_pjrt.so"
    lib = ctypes.CDLL(so_path)
    if not hasattr(lib, "axon_start_nrt_profile"):
        return
    lib.axon_start_nrt_profile.argtypes = [ctypes.POINTER(ctypes.c_int64),
                                           ctypes.c_size_t]
    lib.axon_start_nrt_profile.restype = ctypes.c_int64
    lib.axon_stop_nrt_profile.argtypes = [ctypes.c_char_p]
    lib.axon_stop_nrt_profile.restype = ctypes.c_int64

    @contextlib.contextmanager
    def _hook(output_dir, device_ids):
        import jax
        jax.devices()
        if device_ids:
            ids = (ctypes.c_int64 * len(device_ids))(*device_ids)
            rc = lib.axon_start_nrt_profile(ids, len(device_ids))
        else:
            rc = lib.axon_start_nrt_profile(None, 0)
        if rc != 0:
            raise RuntimeError(f"axon_start_nrt_profile rc={rc}")
        try:
            yield
        finally:
            n = lib.axon_stop_nrt_profile(str(output_dir).encode())
            print(f"ntff profile: {n} file(s) -> {output_dir}",
                  file=sys.stderr)

    mod = types.ModuleType("antenv.axon_hooks")
    mod.get_axon_ntff_profile_hook = lambda: _hook
    mod.set_axon_ntff_profile_hook = lambda h: None
    sys.modules["antenv.axon_hooks"] = mod


# revision 13
# speedup vs baseline: 1.2750x; 1.2750x over previous
"""Trainium2 Bass kernel for a GAT-style attention head.

Reference computation (B=1, C=512, N=8192, F=256):
    seq_fts = einsum('bcn,fc->bfn', x, W1)                  # [1,F,N]
    f1 = seq_fts . w21 + b21 ;  f2 = seq_fts . w22 + b22    # [1,N]
    logits[i,j] = f1[j] + f2[i]  masked by adj>0 (else -1e9)
    logits = leaky_relu(logits, 0.01)
    coefs = softmax(logits, axis=1)        # normalises over i for each j
    ret[i,f] = sum_j coefs[i,j]*seq_fts[f,j] + bias[f]
    out = elu(ret).transpose -> [1,F,N]

Distribution: shard rows i across 8 NeuronCores (1024 rows each).  The
softmax denominator D[j] = sum_i E[i,j] (E = exp of masked leaky-relu
logits) is indexed by the *contracted* axis j, so each core computes a
partial D over its rows, a 32KB AllReduce produces the full D, and 1/D
is folded into the seq_fts columns before the local matmul
    out[f, i_blk] = sum_j (seqT[j,f]/D[j]) * E[j, i_blk].

Per-core pipeline (E kept transposed, [j on partitions, i free], fp16):
  - seqT (=seq_fts^T) and f1 computed on the core's own n-block in fp32,
    then AllGathered (seqT as fp16).
  - logits tile [128j, 1024i]: ACT Lrelu(f2_bcast + f1[j] per-partition
    bias) -> fp16; DVE adds the {0,-1e4} additive mask tile (equivalent
    to the reference's -1e9 pre-relu mask: exp underflows to 0 either
    way); ACT Exp with accum_out giving the partial D for those 128 j.
  - 4-way chunked AllReduce of D overlaps the elementwise phase; fp16
    matmuls accumulate out[f,i] over the 64 j-tiles in PSUM.
  - epilogue: bias add + ELU via relu(x)+exp(min(x,0))-1.
"""

import os
import sys

if "/opt/trn_rl_repo" not in sys.path:
    sys.path.insert(0, "/opt/trn_rl_repo")

import numpy as np

import concourse.bass as bass
import concourse.tile as tile
from concourse import bacc, mybir

F32 = mybir.dt.float32
F16 = mybir.dt.float16

B, C, N, F = 1, 512, 8192, 256
NCORES = 8
NB = N // NCORES          # rows per core (i block)
P = 128
NJT = N // P              # 64 j tiles
NS = NB // P              # 8 n sub-tiles per core
CO = C // P               # 4 contraction tiles for seq_fts
NCHUNK = 4                # allreduce chunks
CH = [20, 20, 20, 4]      # j-tiles per chunk (small tail chunk)
CH0 = [0, 20, 40, 60]     # chunk start offsets
CJT = 20                  # max chunk size (tile sizing)
MM_N = 512                # max moving free dim for 16-bit matmul

AF = mybir.ActivationFunctionType
OP = mybir.AluOpType

_PROGRAM_CACHE = {}
LAST_RESULTS = None       # BassKernelResults of the most recent run (for test.py)


def _build_program(b21f: float, b22f: float):
    nc = bacc.Bacc("TRN2", target_bir_lowering=False, debug=False,
                   num_devices=NCORES)

    # ---- per-core external inputs -------------------------------------
    xb_t = nc.dram_tensor("xb", [C, NB], F32, kind="ExternalInput")
    w1t_t = nc.dram_tensor("w1t", [C, F], F32, kind="ExternalInput")
    w21_t = nc.dram_tensor("w21", [1, F], F32, kind="ExternalInput")
    w22_t = nc.dram_tensor("w22", [1, F], F32, kind="ExternalInput")
    bias_t = nc.dram_tensor("bias", [F], F32, kind="ExternalInput")
    id_t = nc.dram_tensor("ident", [P, P], F32, kind="ExternalInput")
    mk_t = nc.dram_tensor("mk", [N, NB], F16, kind="ExternalInput")
    out_t = nc.dram_tensor("outb", [F, NB], F32, kind="ExternalOutput")

    groups = [list(range(NCORES))]

    with tile.TileContext(nc) as tc:
        with tc.tile_pool(name="dram", bufs=1, space="DRAM") as dram:
            ag1_in = dram.tile([NB], F32, name="ag1_in")
            ag1_out = dram.tile([N], F32, name="ag1_out", addr_space="Shared")
            ag2_in = dram.tile([NB * F], F16, name="ag2_in")
            ag2_out = dram.tile([N * F], F16, name="ag2_out",
                                addr_space="Shared")
            f2tmp = dram.tile([NB], F32, name="f2tmp")
            ar_in = [dram.tile([P * CH[k]], F32, name=f"ar_in{k}")
                     for k in range(NCHUNK)]
            ar_out = [dram.tile([P * CH[k]], F32, name=f"ar_out{k}",
                                addr_space="Shared") for k in range(NCHUNK)]

            # ---------- persistent SBUF ----------
            with tc.tile_pool(name="persist", bufs=1) as persist:
                seqt = persist.tile([P, NJT, F], F16, name="seqt")
                f2b = persist.tile([P, NB], F32, name="f2b")
                f1col = persist.tile([P, NJT], F32, name="f1col")
                bias_sb = persist.tile([P, F // P], F32, name="bias_sb")
                ident = persist.tile([P, P], F32, name="ident")

                # ---------- phase 0 ----------
                with tc.tile_pool(name="p0", bufs=1) as p0, \
                     tc.tile_pool(name="p0ps", bufs=2, space="PSUM") as p0ps:
                    x_sb = p0.tile([P, CO, NB], F32, name="x_sb")
                    nc.sync.dma_start(
                        x_sb[:],
                        xb_t.ap().rearrange("(co ci) n -> ci co n", ci=P))
                    w1t_sb = p0.tile([P, CO, F], F32, name="w1t_sb")
                    nc.sync.dma_start(
                        w1t_sb[:],
                        w1t_t.ap().rearrange("(co ci) f -> ci co f", ci=P))
                    w21b = p0.tile([P, F], F32, name="w21b")
                    nc.sync.dma_start(w21b[:],
                                      w21_t.ap()[0:1, :].to_broadcast((P, F)))
                    w22b = p0.tile([P, F], F32, name="w22b")
                    nc.sync.dma_start(w22b[:],
                                      w22_t.ap()[0:1, :].to_broadcast((P, F)))
                    nc.sync.dma_start(
                        bias_sb[:],
                        bias_t.ap().rearrange("(ft fi) -> fi ft", fi=P))
                    nc.sync.dma_start(ident[:], id_t.ap())

                    # u1/u2 = W1^T w21 / w22  (fp32, c on partitions)
                    u_sb = p0.tile([P, CO, 2], F32, name="u_sb")
                    for co in range(CO):
                        tu = p0.tile([P, F], F32, name="tu", tag="tu")
                        nc.vector.tensor_tensor(tu[:], w1t_sb[:, co, :],
                                                w21b[:], OP.mult)
                        nc.vector.tensor_reduce(u_sb[:, co, 0:1], tu[:],
                                                mybir.AxisListType.X, OP.add)
                        tv = p0.tile([P, F], F32, name="tv", tag="tv")
                        nc.vector.tensor_tensor(tv[:], w1t_sb[:, co, :],
                                                w22b[:], OP.mult)
                        nc.vector.tensor_reduce(u_sb[:, co, 1:2], tv[:],
                                                mybir.AxisListType.X, OP.add)

                    # f1/f2 on own block: u^T x_own (exact fp32 matmuls)
                    f1ps = p0ps.tile([1, NB], F32, name="f1ps", bufs=1)
                    f2ps = p0ps.tile([1, NB], F32, name="f2ps", bufs=1)
                    for ih in range(2):
                        for co in range(CO):
                            nc.tensor.matmul(
                                f1ps[:, ih * MM_N:(ih + 1) * MM_N],
                                lhsT=u_sb[:, co, 0:1],
                                rhs=x_sb[:, co, ih * MM_N:(ih + 1) * MM_N],
                                start=(co == 0), stop=(co == CO - 1))
                    for ih in range(2):
                        for co in range(CO):
                            nc.tensor.matmul(
                                f2ps[:, ih * MM_N:(ih + 1) * MM_N],
                                lhsT=u_sb[:, co, 1:2],
                                rhs=x_sb[:, co, ih * MM_N:(ih + 1) * MM_N],
                                start=(co == 0), stop=(co == CO - 1))
                    f1row = p0.tile([1, NB], F32, name="f1row")
                    nc.vector.tensor_scalar_add(f1row[:], f1ps[:], b21f)
                    nc.sync.dma_start(ag1_in[:].rearrange("n -> () n"),
                                      f1row[:])
                    f2row = p0.tile([1, NB], F32, name="f2row")
                    nc.vector.tensor_scalar_add(f2row[:], f2ps[:], b22f)
                    nc.sync.dma_start(f2tmp[:].rearrange("n -> () n"),
                                      f2row[:])
                    nc.sync.dma_start(
                        f2b[:],
                        f2tmp[None, :].to_broadcast((P, NB)))

                    # small AllGather: f1 (32KB total) — first collective
                    nc.gpsimd.collective_compute(
                        "AllGather", OP.bypass, replica_groups=groups,
                        ins=[ag1_in.opt()], outs=[ag1_out.opt()])

                    # f1col[jp, jt] = f1[jt*128+jp] via PE transpose
                    t64 = p0.tile([NJT, P], F32, name="t64")
                    nc.sync.dma_start(
                        t64[:], ag1_out.rearrange("(jt jp) -> jt jp", jp=P))
                    tps = p0ps.tile([P, NJT], F32, name="tps", bufs=1)
                    nc.tensor.matmul(tps[:], lhsT=t64[:],
                                     rhs=ident[:NJT, :NJT],
                                     is_transpose=True, start=True, stop=True)
                    nc.scalar.copy(f1col[:], tps[:])

                    # seqT (own block, fp16 matmul is enough: it is stored
                    # fp16 anyway)
                    xho = p0.tile([P, CO, NB], F16, name="xho")
                    nc.vector.tensor_copy(xho[:], x_sb[:])
                    w1h = p0.tile([P, CO, F], F16, name="w1h")
                    nc.vector.tensor_copy(w1h[:], w1t_sb[:])
                    seqtown = p0.tile([P, NS, F], F16, name="seqtown")
                    for ns in range(NS):
                        sps = p0ps.tile([P, F], F32, name="sps", tag="sps")
                        for co in range(CO):
                            nc.tensor.matmul(
                                sps[:],
                                lhsT=xho[:, co, ns * P:(ns + 1) * P],
                                rhs=w1h[:, co, :],
                                start=(co == 0), stop=(co == CO - 1))
                        nc.vector.tensor_copy(seqtown[:, ns, :], sps[:])
                    nc.sync.dma_start(
                        ag2_in.rearrange("(ci ns f) -> ci ns f",
                                         ci=P, ns=NS),
                        seqtown[:])
                    nc.gpsimd.collective_compute(
                        "AllGather", OP.bypass, replica_groups=groups,
                        ins=[ag2_in.opt()], outs=[ag2_out.opt()])
                    for b in range(NCORES):
                        srcv = ag2_out.rearrange(
                            "(b ci ns f) -> b ci ns f", b=NCORES, ci=P, ns=NS)
                        nc.sync.dma_start(seqt[:, b * NS:(b + 1) * NS, :],
                                          srcv[b])

                # ---------- main loop ----------
                with tc.tile_pool(name="etpool", bufs=1) as etp, \
                     tc.tile_pool(name="stream", bufs=3) as stream, \
                     tc.tile_pool(name="dtiles", bufs=1) as dtiles, \
                     tc.tile_pool(name="outps", bufs=1, space="PSUM") as outps, \
                     tc.tile_pool(name="epil", bufs=1) as epil:

                    et_c = [etp.tile([P, CH[k], NB], F16, name=f"et{k}",
                                     tag=f"et{k}") for k in range(NCHUNK)]
                    out_ps = [outps.tile([P, MM_N], F32, name=f"out_ps{q}",
                                         tag=f"out_ps{q}")
                              for q in range(4)]
                    dp_c = [dtiles.tile([P, CH[k]], F32, name=f"dp{k}",
                                        tag=f"dp{k}") for k in range(NCHUNK)]
                    inv_c = [dtiles.tile([P, CH[k]], F32, name=f"inv{k}",
                                         tag=f"inv{k}") for k in range(NCHUNK)]

                    for k in range(NCHUNK):
                        for jl in range(CH[k]):
                            jt = CH0[k] + jl
                            mk_tile = stream.tile([P, NB], F16, name="mk_tile",
                                                  tag="mk")
                            nc.sync.dma_start(
                                mk_tile[:],
                                mk_t.ap()[jt * P:(jt + 1) * P, :])
                            if jt % 2 == 0:
                                # ACT path: fused add+leaky-relu on ScalarE
                                ltile = stream.tile([P, NB], F16,
                                                    name="ltile", tag="lt", bufs=2)
                                nc.scalar.activation(
                                    ltile[:], f2b[:], AF.Prelu,
                                    bias=f1col[:, jt:jt + 1], scale=1.0,
                                    alpha=0.01)
                            else:
                                # DVE path: v fp32 exact, then STT leaky-relu
                                vt = stream.tile([P, NB], F32, name="vt",
                                                 tag="vt", bufs=2)
                                nc.vector.tensor_scalar_add(
                                    vt[:], f2b[:], f1col[:, jt:jt + 1])
                                ltile = stream.tile([P, NB], F16,
                                                    name="ltile2", tag="lt2", bufs=2)
                                nc.vector.scalar_tensor_tensor(
                                    ltile[:], vt[:], 0.01, vt[:],
                                    OP.mult, OP.max)
                            l2 = stream.tile([P, NB], F16, name="l2", tag="l2")
                            nc.vector.tensor_tensor(l2[:], ltile[:],
                                                    mk_tile[:], OP.add)
                            nc.scalar.activation(
                                et_c[k][:, jl, :], l2[:], AF.Exp,
                                accum_out=dp_c[k][:, jl:jl + 1])

                        # chunk-k AllReduce of partial D
                        nc.sync.dma_start(
                            ar_in[k].rearrange("(jp jl) -> jp jl", jp=P),
                            dp_c[k][:])
                        nc.gpsimd.collective_compute(
                            "AllReduce", OP.add, replica_groups=groups,
                            ins=[ar_in[k].opt()], outs=[ar_out[k].opt()])
                        dsum = dtiles.tile([P, CH[k]], F32, name=f"dsum{k}",
                                           tag=f"dsum{k}")
                        nc.sync.dma_start(
                            dsum[:],
                            ar_out[k].rearrange("(jp jl) -> jp jl", jp=P))
                        nc.vector.reciprocal(inv_c[k][:], dsum[:])

                        # matmuls for this chunk
                        for jl in range(CH[k]):
                            jt = CH0[k] + jl
                            seqd = stream.tile([P, F], F16, name="seqd",
                                               tag="seqd")
                            nc.vector.tensor_scalar_mul(
                                seqd[:], seqt[:, jt, :],
                                inv_c[k][:, jl:jl + 1])
                            for fi in range(2):
                                for ih in range(2):
                                    nc.tensor.matmul(
                                        out_ps[fi * 2 + ih][:],
                                        lhsT=seqd[:, fi * P:(fi + 1) * P],
                                        rhs=et_c[k][:, jl,
                                               ih * MM_N:(ih + 1) * MM_N],
                                        start=(jt == 0), stop=(jt == NJT - 1))

                    # ---------- epilogue: bias + ELU ----------
                    for fi in range(2):
                        for ih in range(2):
                            ps = out_ps[fi * 2 + ih]
                            t = epil.tile([P, MM_N], F32, name="t", tag="ep_t")
                            nc.scalar.activation(t[:], ps[:], AF.Identity,
                                                 bias=bias_sb[:, fi:fi + 1],
                                                 scale=1.0)
                            r = epil.tile([P, MM_N], F32, name="r", tag="ep_r")
                            nc.vector.tensor_scalar_max(r[:], t[:], 0.0)
                            m = epil.tile([P, MM_N], F32, name="m", tag="ep_m")
                            nc.vector.tensor_scalar_min(m[:], t[:], 0.0)
                            e = epil.tile([P, MM_N], F32, name="e", tag="ep_e")
                            nc.scalar.activation(e[:], m[:], AF.Exp)
                            o = epil.tile([P, MM_N], F32, name="o", tag="ep_o")
                            nc.vector.scalar_tensor_tensor(
                                o[:], e[:], -1.0, r[:], OP.add, OP.add)
                            nc.sync.dma_start(
                                out_t.ap()[fi * P:(fi + 1) * P,
                                           ih * MM_N:(ih + 1) * MM_N], o[:])

    nc.compile()
    return nc


def make_ident_f32(nc, ident):
    nc.gpsimd.memset(ident[:], 0.0)
    nc.gpsimd.affine_select(
        out=ident[:], in_=ident[:], compare_op=OP.not_equal, fill=1.0,
        base=0, pattern=[[-1, P]], channel_multiplier=1)


def kernel(x, adj, W1, w21, b21, w22, b22, bias):
    global LAST_RESULTS
    from concourse.bass_utils import run_bass_kernel_spmd

    x = np.asarray(x)
    adj = np.asarray(adj)
    W1 = np.asarray(W1, dtype=np.float32)
    w21 = np.asarray(w21, dtype=np.float32)
    w22 = np.asarray(w22, dtype=np.float32)
    bias = np.asarray(bias, dtype=np.float32)
    b21f = float(np.asarray(b21))
    b22f = float(np.asarray(b22))

    key = (b21f, b22f)
    if key not in _PROGRAM_CACHE:
        _PROGRAM_CACHE[key] = _build_program(b21f, b22f)
    nc = _PROGRAM_CACHE[key]

    w1t = np.ascontiguousarray(W1.T)                      # [C, F]
    identity = np.eye(P, dtype=np.float32)
    in_maps = []
    for c in range(NCORES):
        blk = slice(c * NB, (c + 1) * NB)
        xb = np.ascontiguousarray(x[0, :, blk], dtype=np.float32)
        # additive mask, transposed: [j, i_local] fp16 {0, -1e4}
        mk = ((adj[0, blk, :].T.astype(np.float32) - 1.0)
              * 1.0e4).astype(np.float16)
        in_maps.append({
            "xb": xb,
            "ident": identity,
            "w1t": w1t,
            "w21": w21.reshape(1, F),
            "w22": w22.reshape(1, F),
            "bias": bias,
            "mk": mk,
        })

    trace = os.environ.get("BASS_KERNEL_TRACE") == "1"
    kwargs = {}
    if trace:
        _install_ntff_hook()
        import concourse.bass_utils as bu
        bu.upload_artifacts = lambda d: d          # no S3 in this sandbox
        kwargs = dict(trace=True, trace_cores=list(range(NCORES)),
                      tmpdir=os.environ.get("BASS_KERNEL_TRACE_DIR"))

    res = run_bass_kernel_spmd(nc, in_maps, core_ids=list(range(NCORES)),
                               **kwargs)
    LAST_RESULTS = res

    out = np.empty((B, F, N), dtype=np.float32)
    for c in range(NCORES):
        out[0, :, c * NB:(c + 1) * NB] = res.results[c]["outb"]
    return out


def _install_ntff_hook():
    """Register the axon NTFF profiling hook (missing antenv.axon_hooks)."""
    import types
    import contextlib
    import ctypes

    if "antenv.axon_hooks" in sys.modules:
        return
    so_path = "/opt/axon/libaxon_pjrt.so"
    lib = ctypes.CDLL(so_path)
    if not hasattr(lib, "axon_start_nrt_profile"):
        return
    lib.axon_start_nrt_profile.argtypes = [ctypes.POINTER(ctypes.c_int64),
                                           ctypes.c_size_t]
    lib.axon_start_nrt_profile.restype = ctypes.c_int64
    lib.axon_stop_nrt_profile.argtypes = [ctypes.c_char_p]
    lib.axon_stop_nrt_profile.restype = ctypes.c_int64

    @contextlib.contextmanager
    def _hook(output_dir, device_ids):
        import jax
        jax.devices()
        if device_ids:
            ids = (ctypes.c_int64 * len(device_ids))(*device_ids)
            rc = lib.axon_start_nrt_profile(ids, len(device_ids))
        else:
            rc = lib.axon_start_nrt_profile(None, 0)
        if rc != 0:
            raise RuntimeError(f"axon_start_nrt_profile rc={rc}")
        try:
            yield
        finally:
            n = lib.axon_stop_nrt_profile(str(output_dir).encode())
            print(f"ntff profile: {n} file(s) -> {output_dir}",
                  file=sys.stderr)

    mod = types.ModuleType("antenv.axon_hooks")
    mod.get_axon_ntff_profile_hook = lambda: _hook
    mod.set_axon_ntff_profile_hook = lambda h: None
    sys.modules["antenv.axon_hooks"] = mod
